# revision 4
# baseline (speedup 1.0000x reference)
"""Trainium2 Bass kernel for nn_CentralAttention1 (sparse_attention), v3.

v3 changes vs v2:
  - conv stream: agent 2 runs solo first, then agents 0+1 interleaved
    per tile in weight-major order (each stationary weight loaded once
    serves 2 back-to-back matmuls; same-weight MMs skip the PE
    drain/reload penalty: ~221ns vs ~275ns measured)
  - fc1 consumes o2 tiles 2 tiles behind conv2 (eviction latency hidden)
  - BatchNorm stats: per-agent obs/acts partials computed early (hidden
    under the stream); feats sums come free via accum_out on the feats
    eviction; shorter serial chain into the AllReduce
  - one act-table load pre-stream (sqrt set); the sigmoid table load is
    issued right after the BN sqrt so it hides under the first post MMs
  - Lrelu activation fuses bias+leaky into one scalar op (sa/vals)
  - post phase: MLP layer2/out emitted weight-major (3-agent adjacency)
  - head: agent-2 scan + first conv weights DMA'd first in fine chunks
"""

import os
import numpy as np
import ml_dtypes

import concourse.bass as bass
import concourse.bacc as bacc
import concourse.tile as tile
from concourse import mybir
from concourse.bass_utils import run_bass_kernel_spmd

# ---- problem sizes (hardcoded per the task spec) ----
NAG, B, H, HEADS, AD = 3, 4096, 128, 8, 16
STATE, ACTD, SCAN, OUTF, HID = 48, 2, 256, 10, 256
EPS = 1e-5
NCORES = 8
BL = B // NCORES            # 512 rows per agent per core
R = NAG * BL                # 1536 rows per core
NB = BL                     # free-dim block = one agent block
P2 = 250                    # conv2 output positions
QT = 63                     # conv tiles of 4 positions (252 = 63*4)
NTOT = NAG * B

F32 = mybir.dt.float32
BF16 = mybir.dt.bfloat16
BF16NP = ml_dtypes.bfloat16
AX = mybir.AxisListType.X
AF = mybir.ActivationFunctionType
OP = mybir.AluOpType
LRELU = os.environ.get("V3_LRELU", "1") == "1"
ACCUM = os.environ.get("V3_ACCUM", "1") == "1"
SYNCDMA = os.environ.get("V3_SYNCDMA", "1") == "1"
TTR = os.environ.get("V3_TTR", "0") == "1"


def _sq_stats(nc, scr_ap, in_ap, accum_ap):
    """accum_ap = row-sums of in_ap**2 (scr_ap is scratch, same shape)."""
    if TTR:
        nc.vector.tensor_tensor_reduce(
            out=scr_ap, in0=in_ap, in1=in_ap,
            scale=1.0, scalar=0.0, op0=OP.mult, op1=OP.add,
            accum_out=accum_ap)
    else:
        nc.vector.tensor_mul(scr_ap, in_ap, in_ap)
        nc.vector.reduce_sum(accum_ap, scr_ap, axis=AX)


def _leaky_evict(nc, wpool, dst, src_ps, bias_ap):
    """dst = leaky_relu(src_ps + bias, 0.01), fused on ACT if available."""
    if LRELU:
        nc.scalar.activation(dst, src_ps, AF.Prelu, bias=bias_ap, alpha=0.01)
    else:
        tmp = wpool.tile([128, NB], F32, tag="lk")
        nc.scalar.activation(tmp[:], src_ps, AF.Identity, bias=bias_ap)
        nc.vector.scalar_tensor_tensor(
            out=dst, in0=tmp[:], scalar=0.01, in1=tmp[:],
            op0=OP.mult, op1=OP.max)


def _t1_parts(q):
    """conv1 tile q -> list of (t1 stack index, scan block index)."""
    if q <= 30:
        return [(q, 0)]
    if q == 31:
        return [(31, 0), (32, 1)]
    return [(q - 32, 1)]


def build_program():
    nc = bacc.Bacc(num_devices=NCORES)

    scan_t = nc.dram_tensor("scan_t", [SCAN, R], BF16, kind="ExternalInput")
    obs_t = nc.dram_tensor("obs_t", [STATE, R], BF16, kind="ExternalInput")
    acts_t = nc.dram_tensor("acts_t", [ACTD, R], BF16, kind="ExternalInput")
    t1_d = nc.dram_tensor("t1", [128, 33, 128], BF16, kind="ExternalInput")
    t2_d = nc.dram_tensor("t2", [128, 320], BF16, kind="ExternalInput")
    fc1w_d = nc.dram_tensor("fc1w", [128, QT, 256], BF16, kind="ExternalInput")
    fc2w_d = nc.dram_tensor("fc2w", [128, 2, 16], BF16, kind="ExternalInput")
    encw_d = nc.dram_tensor("encw", [128, 128], BF16, kind="ExternalInput")
    attw_d = nc.dram_tensor("attw", [128, 5, 128], BF16, kind="ExternalInput")
    hsum_d = nc.dram_tensor("hsum", [128, 8], BF16, kind="ExternalInput")
    hbc_d = nc.dram_tensor("hbc", [8, 128], BF16, kind="ExternalInput")
    mlpw_d = nc.dram_tensor("mlpw", [128, 2, 1152], BF16, kind="ExternalInput")
    bias_d = nc.dram_tensor("bias", [128, 20], F32, kind="ExternalInput")
    out_d = nc.dram_tensor("out", [2, R], F32, kind="ExternalOutput")

    with tile.TileContext(nc) as tc:
        with (
            tc.tile_pool(name="dram", bufs=1, space="DRAM") as dram,
            tc.tile_pool(name="cst", bufs=1) as cst,
            tc.tile_pool(name="ypool", bufs=7) as ypool,
            tc.tile_pool(name="opool", bufs=7) as opool,
            tc.tile_pool(name="xpool", bufs=4) as xpool,
            tc.tile_pool(name="qpool", bufs=3) as qpool,
            tc.tile_pool(name="othp", bufs=3) as othp,
            tc.tile_pool(name="attp", bufs=12) as attp,
            tc.tile_pool(name="lkp", bufs=3) as lkp,
            tc.tile_pool(name="mlph", bufs=26) as mlph,
        ):
            # ---- weight / input DMAs (program order ~ priority) ----
            # critical set first: bias, agent-2 scan, first conv weights
            biasb = cst.tile([128, 20], F32, tag="bias")
            nc.sync.dma_start(out=biasb, in_=bias_d[:])
            s0 = cst.tile([128, R], BF16, tag="s0")
            s1 = cst.tile([128, R], BF16, tag="s1")
            c2 = bass.ts(2, NB)
            for p in range(0, 128, 32):
                nc.sync.dma_start(out=s0[p:p + 32, c2],
                                  in_=scan_t[p:p + 32, c2])
            t1c = []
            for k in range(4):
                n = 9 if k < 3 else 6
                t = cst.tile([128, n, 128], BF16, tag=f"t1c{k}")
                t1c.append(t)
            for j in range(3):
                nc.sync.dma_start(out=t1c[0][:, j:j + 1, :],
                                  in_=t1_d[:, j:j + 1, :])
            t2sb = cst.tile([128, 320], BF16, tag="t2")
            nc.sync.dma_start(out=t2sb[:, 0:160], in_=t2_d[:, 0:160])
            nc.sync.dma_start(out=t2sb[:, 160:320], in_=t2_d[:, 160:320])
            fc1c = []
            for k in range(8):
                n = 8 if k < 7 else 7
                t = cst.tile([128, n, 256], BF16, tag=f"fc1c{k}")
                fc1c.append(t)
            for j in range(4):
                nc.sync.dma_start(out=fc1c[0][:, j:j + 1, :],
                                  in_=fc1w_d[:, j:j + 1, :])
            for j in range(3, 9):
                nc.sync.dma_start(out=t1c[0][:, j:j + 1, :],
                                  in_=t1_d[:, j:j + 1, :])
            for p in range(0, 128, 64):
                nc.sync.dma_start(out=s1[p:p + 64, c2],
                                  in_=scan_t[128 + p:128 + p + 64, c2])
            nc.sync.dma_start(out=fc1c[0][:, 4:8, :], in_=fc1w_d[:, 4:8, :])
            for k in range(1, 4):
                n = 9 if k < 3 else 6
                nc.sync.dma_start(out=t1c[k], in_=t1_d[:, 9 * k:9 * k + n, :])
            fc2w = cst.tile([128, 2, 16], BF16, tag="fc2w")
            nc.sync.dma_start(out=fc2w, in_=fc2w_d[:])
            nc.sync.dma_start(out=fc1c[1], in_=fc1w_d[:, 8:16, :])
            # agents 0/1 scan while agent-2 stream runs
            nc.sync.dma_start(out=s0[:, 0:NB], in_=scan_t[0:128, 0:NB])
            nc.sync.dma_start(out=s0[:, NB:2 * NB],
                              in_=scan_t[0:128, NB:2 * NB])
            nc.sync.dma_start(out=s1[:, 0:NB], in_=scan_t[128:256, 0:NB])
            nc.sync.dma_start(out=s1[:, NB:2 * NB],
                              in_=scan_t[128:256, NB:2 * NB])
            for k in range(2, 8):
                n = 8 if k < 7 else 7
                nc.sync.dma_start(out=fc1c[k], in_=fc1w_d[:, 8 * k:8 * k + n, :])
            # BN feature rows in 32-aligned groups: obs 0:48, feats 64:74,
            # acts 96:98; everything else memset to zero.
            inps = cst.tile([128, R], BF16, tag="inps")
            nc.vector.memset(inps[:], 0.0)
            nc.sync.dma_start(out=inps[0:STATE, :], in_=obs_t[:])
            nc.sync.dma_start(out=inps[96:96 + ACTD, :], in_=acts_t[:])
            encw = cst.tile([128, 128], BF16, tag="encw")
            nc.sync.dma_start(out=encw, in_=encw_d[:])
            attw = cst.tile([128, 5, 128], BF16, tag="attw")
            nc.sync.dma_start(out=attw, in_=attw_d[:])
            hsum = cst.tile([128, 8], BF16, tag="hsum")
            nc.sync.dma_start(out=hsum, in_=hsum_d[:])
            hbc = cst.tile([8, 128], BF16, tag="hbc")
            nc.sync.dma_start(out=hbc, in_=hbc_d[:])
            mlpw = cst.tile([128, 2, 1152], BF16, tag="mlpw")
            nc.sync.dma_start(out=mlpw, in_=mlpw_d[:])

            # ---- warm-up: one act-table load (sqrt set) + dummy CC ----
            scr = cst.tile([128, 2], F32, tag="scr")
            nc.vector.memset(scr[:], 0.0)
            scr2 = cst.tile([128, 1], F32, tag="scr2")
            nc.scalar.activation(scr2[:], scr[:, 0:1], AF.Sqrt)
            nc.scalar.activation(scr2[:], scr[:, 0:1], AF.Relu)
            ccd_in = dram.tile([1, 2], F32, tag="ccd_in")
            ccd_out = dram.tile([1, 2], F32, tag="ccd_out",
                                addr_space="Shared")
            nc.gpsimd.dma_start(out=ccd_in[:], in_=scr[0:1, :])
            nc.gpsimd.collective_compute(
                "AllReduce", OP.add,
                replica_groups=[list(range(NCORES))],
                ins=[ccd_in.opt()], outs=[ccd_out.opt()])

            saT = cst.tile([128, NAG, NB], BF16, tag="saT")
            keysT = cst.tile([128, NAG, NB], BF16, tag="keysT")
            valsT = cst.tile([128, NAG, NB], BF16, tag="valsT")
            outq1 = cst.tile([1, R], F32, tag="outq1")
            outq2 = cst.tile([1, R], F32, tag="outq2")
            # statsP/statsQ: cols 0-2 = per-agent early/full partials,
            # cols 3-4 = feats-row partials for agents 0/1 (rows 64:74)
            statsP = cst.tile([128, 5], F32, tag="statsP")
            statsQ = cst.tile([128, 5], F32, tag="statsQ")
            nc.vector.memset(statsP[:], 0.0)
            nc.vector.memset(statsQ[:], 0.0)
            sqscr = cst.tile([128, NB], F32, tag="sqscr")
            sqf = cst.tile([128, 2 * NB], F32, tag="sqf")
            stats2 = cst.tile([128, 2], F32, tag="stats2")

            # early obs/acts stats for agents 0,1 (feats rows still zero;
            # runs as soon as obs/acts DMAs land, hidden under the stream)
            for g in (0, 1):
                col = bass.ts(g, NB)
                nc.vector.reduce_sum(statsP[:, g:g + 1], inps[:, col],
                                     axis=AX)
                _sq_stats(nc, sqscr[:], inps[:, col], statsQ[:, g:g + 1])

            def evict_relu(dst, src_ps, bias_ap, use_act):
                if use_act:
                    nc.scalar.activation(dst, src_ps, AF.Relu, bias=bias_ap)
                else:
                    nc.vector.tensor_scalar(
                        out=dst, in0=src_ps, scalar1=bias_ap, scalar2=0.0,
                        op0=OP.add, op1=OP.max)

            # =========== conv stream (pre-BatchNorm) =========================
            with tc.tile_pool(name="ps_cv", bufs=4, space="PSUM") as ps_cv:
                def conv1(g, q, y_tiles, parity):
                    """conv1 for tile q of agent g -> y_tiles[q]."""
                    col = bass.ts(g, NB)
                    py = ps_cv.tile([128, NB], F32, tag="cnv", name=f"py{g}_{q}")
                    parts = _t1_parts(q)
                    for i, (idx, sb_) in enumerate(parts):
                        src = (s0 if sb_ == 0 else s1)[:, col]
                        nc.tensor.matmul(py, t1c[idx // 9][:, idx % 9, :],
                                         src, start=(i == 0),
                                         stop=(i == len(parts) - 1))
                    yq = ypool.tile([128, NB], BF16, tag="y")
                    evict_relu(yq[:], py[:], biasb[:, 0:1], parity)
                    y_tiles[q] = yq

                def conv1_pair(q, y0, y1, parity):
                    """conv1 tile q for agents 0 and 1, weight-major."""
                    py0 = ps_cv.tile([128, NB], F32, tag="cnv", name=f"pyA{q}")
                    py1 = ps_cv.tile([128, NB], F32, tag="cnv", name=f"pyB{q}")
                    parts = _t1_parts(q)
                    for i, (idx, sb_) in enumerate(parts):
                        src = (s0 if sb_ == 0 else s1)
                        lhs = t1c[idx // 9][:, idx % 9, :]
                        st, sp = (i == 0), (i == len(parts) - 1)
                        nc.tensor.matmul(py0, lhs, src[:, 0:NB], start=st,
                                         stop=sp, skip_group_check=True)
                        nc.tensor.matmul(py1, lhs, src[:, NB:2 * NB],
                                         start=st, stop=sp,
                                         skip_group_check=True)
                    ya = ypool.tile([128, NB], BF16, tag="y")
                    yb = ypool.tile([128, NB], BF16, tag="y")
                    evict_relu(ya[:], py0[:], biasb[:, 0:1], parity)
                    evict_relu(yb[:], py1[:], biasb[:, 0:1], not parity)
                    y0[q] = ya
                    y1[q] = yb

                def conv2(t, y_tiles_list, o2_list, parity):
                    """conv2 tile t for the given agents, weight-major."""
                    nag_ = len(y_tiles_list)
                    pos = [ps_cv.tile([128, NB], F32, tag="cnv",
                                      name=f"po{t}_{i}")
                           for i in range(nag_)]
                    if t < QT - 1:
                        rows, bcol = 128, 1
                        for i in range(nag_):
                            nc.tensor.matmul(pos[i], t2sb[:, 0:128],
                                             y_tiles_list[i][t][:],
                                             start=True, stop=False,
                                             skip_group_check=True)
                        for i in range(nag_):
                            nc.tensor.matmul(pos[i], t2sb[:, 128:256],
                                             y_tiles_list[i][t + 1][:],
                                             start=False, stop=True,
                                             skip_group_check=True)
                    else:
                        rows, bcol = 64, 17
                        for i in range(nag_):
                            nc.tensor.matmul(pos[i][0:64, :],
                                             t2sb[:, 256:320],
                                             y_tiles_list[i][t][:],
                                             start=True, stop=True)
                    for i in range(nag_):
                        o2 = opool.tile([128, NB], BF16, tag="o2")
                        evict_relu(o2[0:rows, :], pos[i][0:rows, :],
                                   biasb[0:rows, bcol:bcol + 1],
                                   (parity + i) % 2 == 0)
                        o2_list[i][t] = o2
                    for i in range(nag_):
                        if t > 0:
                            del y_tiles_list[i][t - 1]
                        if t == QT - 1:
                            del y_tiles_list[i][t]

                def fc1(t, o2_list, fc_ps_list):
                    """fc1 accumulation for tile t, weight-major."""
                    rows = 128 if t < QT - 1 else 64
                    for m in range(2):
                        lhs = fc1c[t // 8][0:rows, t % 8,
                                          128 * m:128 * m + 128]
                        for i in range(len(o2_list)):
                            nc.tensor.matmul(
                                fc_ps_list[i][m], lhs,
                                o2_list[i][t][0:rows, :],
                                start=(t == 0), stop=(t == QT - 1),
                                skip_group_check=True)
                    for i in range(len(o2_list)):
                        del o2_list[i][t]

                def fc1_finish(g, fc_ps, use_feats_accum):
                    """fc1 relu-evict, fc2, feats -> inps; stats partials."""
                    col = bass.ts(g, NB)
                    x3 = []
                    for m in range(2):
                        xm = xpool.tile([128, NB], BF16, tag="x3")
                        if m == 0:
                            nc.scalar.activation(xm[:], fc_ps[m][:], AF.Relu,
                                                 bias=biasb[:, 2:3])
                        else:
                            nc.vector.tensor_scalar(
                                out=xm[:], in0=fc_ps[m][:],
                                scalar1=biasb[:, 3:4], scalar2=0.0,
                                op0=OP.add, op1=OP.max)
                        x3.append(xm)
                    pf = ps_cv.tile([OUTF, NB], F32, tag="cnv",
                                    name=f"pf{g}")
                    for kb in range(2):
                        nc.tensor.matmul(pf, fc2w[:, kb, 0:OUTF], x3[kb][:],
                                         start=(kb == 0), stop=(kb == 1))
                    if use_feats_accum:
                        # agents 0/1 at stream end: feats-row stats on the
                        # least-loaded engine each (entry critical path)
                        if g == 0:
                            # ACT: evict g0; sums+sq via Square accum
                            nc.scalar.activation(inps[64:64 + OUTF, col],
                                                 pf[:], AF.Identity,
                                                 bias=biasb[0:OUTF, 4:5])
                            nc.scalar.activation(
                                sqf[64:64 + OUTF, 0:NB],
                                inps[64:64 + OUTF, col], AF.Square,
                                accum_out=statsQ[64:64 + OUTF, 3:4])
                            nc.vector.reduce_sum(
                                statsP[64:64 + OUTF, 3:4],
                                inps[64:64 + OUTF, col], axis=AX)
                        else:
                            # DVE: evict g1 + sums; gpsimd: sumsq
                            nc.vector.tensor_scalar(
                                out=inps[64:64 + OUTF, col], in0=pf[:],
                                scalar1=biasb[0:OUTF, 4:5], scalar2=None,
                                op0=OP.add)
                            nc.vector.reduce_sum(
                                statsP[64:64 + OUTF, 4:5],
                                inps[64:64 + OUTF, col], axis=AX)
                            nc.scalar.activation(
                                sqf[64:64 + OUTF, NB:2 * NB],
                                inps[64:64 + OUTF, col], AF.Square,
                                accum_out=statsQ[64:64 + OUTF, 4:5])
                    else:
                        # agent 2: full-column stats, hidden under 0/1 stream
                        nc.scalar.activation(inps[64:64 + OUTF, col], pf[:],
                                             AF.Identity,
                                             bias=biasb[0:OUTF, 4:5])
                        nc.vector.reduce_sum(statsP[:, g:g + 1],
                                             inps[:, col], axis=AX)
                        _sq_stats(nc, sqscr[:], inps[:, col],
                                  statsQ[:, g:g + 1])

                # ---------------- agent 2 solo ----------------
                with tc.tile_pool(name="ps_fA", bufs=2,
                                  space="PSUM") as ps_fA:
                    sc2 = nc.named_scope("conv_g2")
                    sc2.__enter__()
                    fcA = [ps_fA.tile([128, NB], F32, tag="pfc",
                                      name=f"pfcA{m}") for m in range(2)]
                    y2, oo2 = {}, {}
                    for q in range(QT):
                        conv1(2, q, y2, q % 2 == 0 and q >= 6)
                        if q >= 1:
                            conv2(q - 1, [y2], [oo2], q)
                        if q >= 2:
                            fc1(q - 2, [oo2], [fcA])
                    conv2(QT - 1, [y2], [oo2], 0)
                    fc1(QT - 2, [oo2], [fcA])
                    fc1(QT - 1, [oo2], [fcA])
                    fc1_finish(2, fcA, use_feats_accum=False)
                    sc2.__exit__(None, None, None)

                # ---------------- agents 0 + 1 interleaved ----------------
                with tc.tile_pool(name="ps_fB", bufs=4,
                                  space="PSUM") as ps_fB:
                    sc01 = nc.named_scope("conv_g01")
                    sc01.__enter__()
                    fcB = [[ps_fB.tile([128, NB], F32, tag="pfc",
                                       name=f"pfcB{g}{m}")
                            for m in range(2)] for g in range(2)]
                    y0, y1 = {}, {}
                    oA, oB = {}, {}
                    for q in range(QT):
                        conv1_pair(q, y0, y1, q % 2 == 0)
                        if q >= 1:
                            conv2(q - 1, [y0, y1], [oA, oB], q)
                        if q >= 2:
                            fc1(q - 2, [oA, oB], [fcB[0], fcB[1]])
                    conv2(QT - 1, [y0, y1], [oA, oB], 0)
                    fc1(QT - 2, [oA, oB], [fcB[0], fcB[1]])
                    fc1(QT - 1, [oA, oB], [fcB[0], fcB[1]])
                    fc1_finish(0, fcB[0], use_feats_accum=True)
                    fc1_finish(1, fcB[1], use_feats_accum=True)
                    sc01.__exit__(None, None, None)

                # ============ BatchNorm statistics + AllReduce ===========
                bn_scope = nc.named_scope("bn")
                bn_scope.__enter__()
                nc.vector.reduce_sum(stats2[:, 0:1], statsP[:], axis=AX)
                nc.vector.reduce_sum(stats2[:, 1:2], statsQ[:], axis=AX)
                cc_in = dram.tile([128, 2], F32, tag="cc_in")
                cc_out = dram.tile([128, 2], F32, tag="cc_out",
                                   addr_space="Shared")
                if SYNCDMA:
                    nc.sync.dma_start(out=cc_in[:], in_=stats2[:])
                else:
                    nc.gpsimd.dma_start(out=cc_in[:], in_=stats2[:])
                nc.gpsimd.collective_compute(
                    "AllReduce", OP.add,
                    replica_groups=[list(range(NCORES))],
                    ins=[cc_in.opt()], outs=[cc_out.opt()])
                gst = cst.tile([128, 2], F32, tag="gst")
                nc.sync.dma_start(out=gst[:], in_=cc_out[:])
                # PE filler: keep the HAM clock warm through the collective
                # (covers entry+algo latency every core pays; fillers have
                # no CC dependency so they run during the wait)
                nfill = int(os.environ.get("V3_FILL", "55"))
                if nfill:
                    fill_ps = [ps_cv.tile([128, NB], F32, tag="cnv",
                                          name=f"fill{i}") for i in range(2)]
                    for i in range(nfill):
                        nc.tensor.matmul(fill_ps[i % 2], t2sb[:, 0:128],
                                         s0[:, 0:NB], start=True, stop=True,
                                         skip_group_check=True)
                    nc.vector.tensor_scalar(out=scr[:, 0:2],
                                            in0=fill_ps[0][:, 0:2],
                                            scalar1=0.0, scalar2=None,
                                            op0=OP.mult)
                    nc.vector.tensor_scalar(out=scr[:, 0:2],
                                            in0=fill_ps[1][:, 0:2],
                                            scalar1=0.0, scalar2=None,
                                            op0=OP.mult)
            # var = gst1/N - (gst0/N)^2; sd = sqrt(var+eps)
            sqmu = cst.tile([128, 1], F32, tag="sqmu")
            nc.scalar.activation(sqmu[:], gst[:, 0:1], AF.Square,
                                 scale=1.0 / NTOT)
            var_ = cst.tile([128, 1], F32, tag="var_")
            nc.vector.scalar_tensor_tensor(
                out=var_[:], in0=gst[:, 1:2], scalar=1.0 / NTOT,
                in1=sqmu[:], op0=OP.mult, op1=OP.subtract)
            sd = cst.tile([128, 1], F32, tag="sd")
            nc.scalar.activation(sd[:], var_[:], AF.Sqrt,
                                 bias=biasb[:, 18:19])
            mu_bf = cst.tile([128, 1], BF16, tag="mu_bf")
            nc.scalar.activation(mu_bf[:], gst[:, 0:1], AF.Identity,
                                 scale=1.0 / NTOT)
            scr3 = cst.tile([1, 1], F32, tag="scr3")
            # fold BN into encoder: encw_s = encw / sd (per contraction
            # row); enc bias' = enc_b - encw_s.T @ mu
            rstd = cst.tile([128, 1], F32, tag="rstd")
            nc.vector.reciprocal(rstd[:], sd[:])
            encw_s = cst.tile([128, 128], BF16, tag="encw_s")
            nc.vector.tensor_scalar(out=encw_s[:], in0=encw[:],
                                    scalar1=rstd[:], scalar2=None,
                                    op0=OP.mult)
            encbias = cst.tile([128, 1], F32, tag="encbias")
            bn_scope.__exit__(None, None, None)

            # ======================= post-BN network =========================
            with (
                tc.tile_pool(name="ps_p", bufs=5, space="PSUM") as ps_p,
                tc.tile_pool(name="ps_l", bufs=1, space="PSUM") as ps_l,
                tc.tile_pool(name="ps_o", bufs=2, space="PSUM") as ps_o,
            ):
                post_scope = nc.named_scope("post")
                post_scope.__enter__()
                pmb = ps_l.tile([128, 1], F32, tag="pl", name="pmb")
                nc.tensor.matmul(pmb, encw_s[:], mu_bf[:],
                                 start=True, stop=True)
                nc.vector.tensor_tensor(out=encbias[:], in0=biasb[:, 5:6],
                                        in1=pmb[:], op=OP.subtract)

                # --- sa = leaky(encw_s @ inps + encbias), all agents ---
                pe_ = {}
                for g in (1, 2, 0):
                    p = ps_p.tile([128, NB], F32, tag="pp", name=f"pe{g}")
                    nc.tensor.matmul(p, encw_s[:], inps[:, bass.ts(g, NB)],
                                     start=True, stop=True)
                    pe_[g] = p
                for g in (1, 2, 0):
                    # fused bias + leaky relu (parametric_relu: every table)
                    _leaky_evict(nc, lkp, saT[:, g, :], pe_[g][:],
                                 encbias[:])


                # --- keys/vals/q; agent order 1,2,0 so that agent 0's
                # attention chain (which needs agents 1+2's keys/vals) can
                # start as early as possible ---
                ORD = (1, 2, 0)
                qT = {}
                pk_, pv_, pq_ = {}, {}, {}
                for g in ORD:
                    pk_[g] = ps_p.tile([128, NB], F32, tag="pp",
                                       name=f"pk{g}")
                    nc.tensor.matmul(pk_[g], attw[:, 0, :], saT[:, g, :],
                                     start=True, stop=True)
                for g in ORD:
                    nc.vector.tensor_scalar(out=keysT[:, g, :],
                                            in0=pk_[g][:], scalar1=0.0,
                                            scalar2=None, op0=OP.add)
                for g in ORD:
                    pv_[g] = ps_p.tile([128, NB], F32, tag="pp",
                                       name=f"pv{g}")
                    nc.tensor.matmul(pv_[g], attw[:, 1, :], saT[:, g, :],
                                     start=True, stop=True)
                for g in (0, 1, 2):
                    pq_[g] = ps_p.tile([128, NB], F32, tag="pp",
                                       name=f"pq{g}")
                    nc.tensor.matmul(pq_[g], attw[:, 2 + g, :], saT[:, g, :],
                                     start=True, stop=True)
                for g in ORD:
                    _leaky_evict(nc, lkp, valsT[:, g, :], pv_[g][:],
                                 biasb[:, 6:7])
                # sigmoid table load after the vals evicts, before the
                # first attention sigmoid needs it
                nc.scalar.activation(scr3[:], sd[0:1, 0:1], AF.Sigmoid)

                def q_evict(g):
                    qg = qpool.tile([128, NB], BF16, tag="qt", name=f"q{g}")
                    nc.vector.tensor_scalar(out=qg[:], in0=pq_[g][:],
                                            scalar1=0.0, scalar2=None,
                                            op0=OP.add)
                    qT[g] = qg

                # dk/dv/prod in dependency-first DVE order: agent 0's
                # chain only needs agents 1+2's keys/vals evictions
                dks, dvs, prods = {}, {}, {}

                def attn_dve(g):
                    oa, ob = [o for o in range(NAG) if o != g]
                    dk = attp.tile([128, NB], BF16, tag="dk", name=f"dk{g}")
                    nc.vector.tensor_sub(dk[:], keysT[:, oa, :],
                                         keysT[:, ob, :])
                    dv = attp.tile([128, NB], BF16, tag="dv", name=f"dv{g}")
                    nc.vector.tensor_sub(dv[:], valsT[:, oa, :],
                                         valsT[:, ob, :])
                    prod = attp.tile([128, NB], BF16, tag="prod",
                                     name=f"pr{g}")
                    nc.vector.tensor_mul(prod[:], qT[g][:], dk[:])
                    dks[g], dvs[g], prods[g] = dk, dv, prod

                q_evict(0)
                attn_dve(0)
                q_evict(1)
                attn_dve(1)
                q_evict(2)
                attn_dve(2)

                # --- attention per agent, MLP layer 1 interleaved ---
                h_cur = {}
                oth = {}

                def mlp_l1(g, net):
                    h_prev = (saT[:, g, :], oth[g][:])
                    h_new = []
                    for mb in range(2):
                        pm = ps_p.tile([128, NB], F32, tag="pp",
                                       name=f"pm{g}_{net}_0_{mb}")
                        for kb in range(2):
                            c0 = kb * 256 + mb * 128
                            nc.tensor.matmul(
                                pm, mlpw[:, net, c0:c0 + 128],
                                h_prev[kb], start=(kb == 0), stop=(kb == 1))
                        hm = mlph.tile([128, NB], BF16, tag="h")
                        bcol = (7 if net == 0 else 12) + mb
                        evict_relu(hm[:], pm[:], biasb[:, bcol:bcol + 1],
                                   (2 * g + net + mb) % 2 == 0)
                        h_new.append(hm)
                    h_cur[(g, net)] = tuple(h_new)

                for g in (0, 1, 2):
                    oa, ob = [o for o in range(NAG) if o != g]
                    dv = dvs[g]
                    pl = ps_l.tile([8, NB], F32, tag="pl")
                    nc.tensor.matmul(pl, hsum[:], prods[g][:],
                                     start=True, stop=True)
                    wa = attp.tile([8, NB], BF16, tag="wa")
                    nc.scalar.activation(wa[:], pl[:], AF.Sigmoid, scale=0.25)
                    pw = ps_p.tile([128, NB], F32, tag="pp", name=f"pw{g}")
                    nc.tensor.matmul(pw, hbc[:], wa[:], start=True, stop=True)
                    m1 = attp.tile([128, NB], F32, tag="m1")
                    nc.vector.tensor_mul(m1[:], pw[:], dv[:])
                    o = othp.tile([128, NB], BF16, tag="oth")
                    nc.vector.tensor_add(o[:], m1[:], valsT[:, ob, :])
                    oth[g] = o
                    mlp_l1(g, 0)
                    mlp_l1(g, 1)

                # --- MLP layer 2 weight-major (3-agent weight adjacency) ---
                h2 = {}
                for net in range(2):
                    pm2 = {}
                    for mb in range(2):
                        for g in range(NAG):
                            pm2[(g, mb)] = ps_p.tile(
                                [128, NB], F32, tag="pp",
                                name=f"pm2_{g}_{net}_{mb}")
                        for kb in range(2):
                            c0 = 512 + kb * 256 + mb * 128
                            lhs = mlpw[:, net, c0:c0 + 128]
                            for g in range(NAG):
                                nc.tensor.matmul(
                                    pm2[(g, mb)], lhs, h_cur[(g, net)][kb][:],
                                    start=(kb == 0), stop=(kb == 1),
                                    skip_group_check=True)
                        for g in range(NAG):
                            hm = mlph.tile([128, NB], BF16, tag="h")
                            bcol = (9 if net == 0 else 14) + mb
                            evict_relu(hm[:], pm2[(g, mb)][:],
                                       biasb[:, bcol:bcol + 1],
                                       (g + net + mb) % 2 == 0)
                            h2[(g, net, mb)] = hm

                # --- output layer, weight-major ---
                po_ = {}
                for net in range(2):
                    for kb in range(2):
                        lhs = mlpw[:, net, 1024 + 64 * kb:1025 + 64 * kb]
                        for g in range(NAG):
                            if kb == 0:
                                po_[(g, net)] = ps_o.tile(
                                    [1, NB], F32, tag="pout",
                                    name=f"po_{g}_{net}")
                            nc.tensor.matmul(
                                po_[(g, net)], lhs, h2[(g, net, kb)][:],
                                start=(kb == 0), stop=(kb == 1),
                                skip_group_check=True)
                for g in range(NAG):
                    col = bass.ts(g, NB)
                    for net in range(2):
                        bcol = 11 if net == 0 else 16
                        dst = (outq1 if net == 0 else outq2)[0:1, col]
                        nc.scalar.activation(dst, po_[(g, net)][:],
                                             AF.Identity,
                                             bias=biasb[0:1, bcol:bcol + 1])
                    nc.sync.dma_start(out=out_d[0:1, col],
                                      in_=outq1[0:1, col])
                    nc.sync.dma_start(out=out_d[1:2, col],
                                      in_=outq2[0:1, col])
                post_scope.__exit__(None, None, None)
    return nc


# ======================= host-side preparation ===========================

def _prep_shared(i):
    f32 = np.float32
    w1 = np.asarray(i["conv_w1"], f32)[:, 0, :]           # [32, 5]
    w2 = np.asarray(i["conv_w2"], f32)                    # [32, 32, 3]
    fw1 = np.asarray(i["fc_w1"], f32)                     # [256, 8000]
    fw2 = np.asarray(i["fc_w2"], f32)                     # [10, 256]
    encw_ = np.asarray(i["enc_w"], f32)                   # [128, 60]
    Wk = np.asarray(i["Wk"], f32)
    Wv = np.asarray(i["Wv"], f32)
    Wq = np.asarray(i["Wq"], f32)

    t1 = np.zeros((128, 33, 128), f32)
    for idx in range(32):
        r0 = 4 * idx if idx < 31 else 124
        for dp in range(4):
            for j in range(5):
                r = r0 + dp + j
                if r < 128:
                    t1[r, idx, dp::4] = w1[:, j]
    for dp in range(4):
        for r in range(4):
            j = r + 4 - dp
            if 0 <= j < 5:
                t1[r, 32, dp::4] = w1[:, j]

    t2 = np.zeros((128, 320), f32)
    for dp in range(4):
        for j in range(3):
            e = dp + j
            for cp in range(32):
                if e < 4:
                    t2[4 * cp + e, dp:128:4] = w2[:, cp, j]
                else:
                    t2[4 * cp + (e - 4), 128 + dp:256:4] = w2[:, cp, j]
    for dp in range(2):
        for j in range(3):
            e = dp + j
            for cp in range(32):
                t2[4 * cp + e, 256 + dp:320:2] = w2[:, cp, j]

    fc1w = np.zeros((128, QT, 256), f32)
    for q in range(QT - 1):
        for dp in range(4):
            p = 4 * q + dp
            fc1w[dp::4, q, :] = fw1[:, p::P2].T
    for dp in range(2):
        fc1w[dp:64:2, QT - 1, :] = fw1[:, (248 + dp)::P2].T

    fc2w = np.zeros((128, 2, 16), f32)
    for kb in range(2):
        fc2w[:, kb, 0:OUTF] = fw2[:, 128 * kb:128 * kb + 128].T

    encw_full = np.zeros((128, 128), f32)
    encw_full[0:STATE, :] = encw_.T[0:STATE, :]            # obs rows
    encw_full[64:64 + OUTF, :] = encw_.T[50:60, :]         # feats rows
    encw_full[96:96 + ACTD, :] = encw_.T[48:50, :]         # acts rows

    attw = np.zeros((128, 5, 128), f32)
    attw[:, 0, :] = Wk.reshape(128, H).T
    attw[:, 1, :] = Wv.reshape(128, H).T
    for g in range(NAG):
        attw[:, 2 + g, :] = Wq[g].reshape(128, H).T

    hsum = np.kron(np.eye(8, dtype=f32), np.ones((16, 1), f32))  # [128, 8]
    hbc = np.ascontiguousarray(hsum.T)                           # [8, 128]

    mlpw = np.zeros((128, 2, 1152), f32)
    for net, pre in enumerate(("q1", "q2")):
        mw1 = np.asarray(i[pre + "_w1"], f32)
        mw2 = np.asarray(i[pre + "_w2"], f32)
        mw3 = np.asarray(i[pre + "_w3"], f32)
        for kb in range(2):
            mlpw[:, net, kb * 256:(kb + 1) * 256] = \
                mw1[:, 128 * kb:128 * kb + 128].T
            mlpw[:, net, 512 + kb * 256:512 + (kb + 1) * 256] = \
                mw2[:, 128 * kb:128 * kb + 128].T
            mlpw[:, net, 1024 + 64 * kb] = mw3[0, 128 * kb:128 * kb + 128]

    bias = np.zeros((128, 20), f32)
    bias[:, 0] = np.repeat(np.asarray(i["conv_b1"], f32), 4)
    bias[:, 1] = np.repeat(np.asarray(i["conv_b2"], f32), 4)
    bias[:, 2] = np.asarray(i["fc_b1"], f32)[0:128]
    bias[:, 3] = np.asarray(i["fc_b1"], f32)[128:256]
    bias[0:OUTF, 4] = np.asarray(i["fc_b2"], f32)
    bias[:, 5] = np.asarray(i["enc_b"], f32)
    bias[:, 6] = np.asarray(i["bv"], f32).reshape(128)
    for net, pre in enumerate(("q1", "q2")):
        b1 = np.asarray(i[pre + "_b1"], f32)
        b2 = np.asarray(i[pre + "_b2"], f32)
        b3 = np.asarray(i[pre + "_b3"], f32)
        c0 = 7 if net == 0 else 12
        bias[:, c0] = b1[0:128]
        bias[:, c0 + 1] = b1[128:256]
        bias[:, c0 + 2] = b2[0:128]
        bias[:, c0 + 3] = b2[128:256]
        bias[0, 11 if net == 0 else 16] = b3[0]
    bias[0:64, 17] = np.repeat(np.asarray(i["conv_b2"], f32), 2)
    bias[:, 18] = EPS

    bf = BF16NP
    return {
        "t1": t1.astype(bf), "t2": t2.astype(bf),
        "fc1w": fc1w.astype(bf), "fc2w": fc2w.astype(bf),
        "encw": encw_full.astype(bf), "attw": attw.astype(bf),
        "hsum": hsum.astype(bf), "hbc": hbc.astype(bf),
        "mlpw": mlpw.astype(bf), "bias": bias,
    }


def _shard(arr, c):
    out = np.empty((R, arr.shape[1]), np.float32)
    for g in range(NAG):
        out[g * BL:(g + 1) * BL] = arr[g * B + c * BL: g * B + (c + 1) * BL]
    return np.ascontiguousarray(out.T).astype(BF16NP)


_CACHE = {}


def _strip_redundant_ldweights(nc):
    """Remove back-to-back InstLdweights with identical weight APs.

    The PE keeps its stationary operand between matmuls; a reload of the
    same weights forces the array to drain first (~50ns/matmul measured).
    Runs pre-finalize (waits are still on the matmuls at this point);
    references to a removed load are remapped to the kept one.
    """
    removed = 0
    mapping = {}
    for f in nc.m.functions:
        for b in f.blocks:
            insts = list(b.instructions)
            out, last_sig, kept_name = [], None, None
            for inst in insts:
                if type(inst).__name__ == 'InstLdweights':
                    sig = (str(inst.ins[0]) + '|' + str(inst.perf_mode) +
                           '|' + str(inst.is_transpose) + '|' +
                           str(inst.tile_position))
                    if sig == last_sig and kept_name is not None:
                        mapping[inst.name] = kept_name
                        removed += 1
                        continue
                    last_sig, kept_name = sig, inst.name
                out.append(inst)
            if len(out) != len(insts):
                b.instructions = out
    if mapping:
        for f in nc.m.functions:
            for b in f.blocks:
                for inst in b.instructions:
                    inst.remap_dependency_names(mapping)
    return removed


def _get_prog():
    if "nc" not in _CACHE:
        nc = build_program()
        if os.environ.get("V3_STRIP", "1") == "1":
            _strip_redundant_ldweights(nc)
        nc.finalize()
        _CACHE["nc"] = nc
    return _CACHE["nc"]


def _make_in_maps(inputs):
    shared = _prep_shared(inputs)
    obs = np.asarray(inputs["obs"], np.float32)
    acts = np.asarray(inputs["acts"], np.float32)
    scan = np.asarray(inputs["scan"], np.float32)
    in_maps = []
    for c in range(NCORES):
        m = dict(shared)
        m["scan_t"] = _shard(scan, c)
        m["obs_t"] = _shard(obs, c)
        m["acts_t"] = _shard(acts, c)
        in_maps.append(m)
    return in_maps


def _gather(results):
    q = np.empty((2, NAG, B), np.float32)
    for c, r in enumerate(results):
        o = np.asarray(r["out"]).reshape(2, NAG, BL)
        q[:, :, c * BL:(c + 1) * BL] = o
    q1 = np.ascontiguousarray(q[0].reshape(NTOT, 1))
    q2 = np.ascontiguousarray(q[1].reshape(NTOT, 1))
    return q1, q2


def kernel(**inputs):
    nc = _get_prog()
    in_maps = _make_in_maps(inputs)
    if os.environ.get("KERNEL_BACKEND") == "sim":
        from concourse import bass_interp
        sim = bass_interp.MultiCoreSim(nc, NCORES)
        for c in range(NCORES):
            for k, v in in_maps[c].items():
                sim.cores[c].tensor(k)[:] = v
        sim.simulate()
        results = [{"out": np.array(sim.cores[c].tensor("out"))}
                   for c in range(NCORES)]
        return _gather(results)
    res = run_bass_kernel_spmd(nc, in_maps, list(range(NCORES)))
    return _gather(res.results)


def kernel_profiled(**inputs):
    """Like kernel() but NTFF-traced; returns ((q1, q2), exec_time_ns)."""
    _install_ntff_hook()
    nc = _get_prog()
    in_maps = _make_in_maps(inputs)
    res = run_bass_kernel_spmd(nc, in_maps, list(range(NCORES)), trace=True)
    return _gather(res.results), res.exec_time_ns


def _install_ntff_hook():
    """Provide antenv.axon_hooks (absent in this image) and register the
    ctypes NTFF profile hook against libaxon_pjrt.so."""
    import sys
    import types
    import ctypes
    import contextlib

    if "antenv.axon_hooks" not in sys.modules:
        mod = types.ModuleType("antenv.axon_hooks")
        mod._hook = None
        mod.set_axon_ntff_profile_hook = lambda h: setattr(mod, "_hook", h)
        mod.get_axon_ntff_profile_hook = lambda: mod._hook
        sys.modules["antenv.axon_hooks"] = mod
        import antenv
        antenv.axon_hooks = mod
    mod = sys.modules["antenv.axon_hooks"]
    if mod.get_axon_ntff_profile_hook() is not None:
        return
    so_path = "/opt/axon/libaxon_pjrt.so"
    lib = ctypes.CDLL(so_path)
    if not hasattr(lib, "axon_start_nrt_profile"):
        return
    lib.axon_start_nrt_profile.argtypes = [
        ctypes.POINTER(ctypes.c_int64), ctypes.c_size_t]
    lib.axon_start_nrt_profile.restype = ctypes.c_int64
    lib.axon_stop_nrt_profile.argtypes = [ctypes.c_char_p]
    lib.axon_stop_nrt_profile.restype = ctypes.c_int64

    @contextlib.contextmanager
    def _hook(output_dir, device_ids):
        import jax
        jax.devices()
        if device_ids:
            ids = (ctypes.c_int64 * len(device_ids))(*device_ids)
            rc = lib.axon_start_nrt_profile(ids, len(device_ids))
        else:
            rc = lib.axon_start_nrt_profile(None, 0)
        if rc != 0:
            raise RuntimeError(f"axon_start_nrt_profile rc={rc}")
        try:
            yield
        finally:
            n = lib.axon_stop_nrt_profile(str(output_dir).encode())
            if n < 0:
                raise RuntimeError(f"axon_stop_nrt_profile rc={n}")

    mod.set_axon_ntff_profile_hook(_hook)


# revision 5
# speedup vs baseline: 1.0731x; 1.0731x over previous
"""Trainium2 Bass kernel for nn_CentralAttention1 (sparse_attention), v3.

v3 changes vs v2:
  - conv stream: agent 2 runs solo first, then agents 0+1 interleaved
    per tile in weight-major order (each stationary weight loaded once
    serves 2 back-to-back matmuls; same-weight MMs skip the PE
    drain/reload penalty: ~221ns vs ~275ns measured)
  - fc1 consumes o2 tiles 2 tiles behind conv2 (eviction latency hidden)
  - BatchNorm stats: per-agent obs/acts partials computed early (hidden
    under the stream); feats sums come free via accum_out on the feats
    eviction; shorter serial chain into the AllReduce
  - one act-table load pre-stream (sqrt set); the sigmoid table load is
    issued right after the BN sqrt so it hides under the first post MMs
  - Lrelu activation fuses bias+leaky into one scalar op (sa/vals)
  - post phase: MLP layer2/out emitted weight-major (3-agent adjacency)
  - head: agent-2 scan + first conv weights DMA'd first in fine chunks
"""

import os
import numpy as np
import ml_dtypes

import concourse.bass as bass
import concourse.bacc as bacc
import concourse.tile as tile
from concourse import mybir
from concourse.bass_utils import run_bass_kernel_spmd

# ---- problem sizes (hardcoded per the task spec) ----
NAG, B, H, HEADS, AD = 3, 4096, 128, 8, 16
STATE, ACTD, SCAN, OUTF, HID = 48, 2, 256, 10, 256
EPS = 1e-5
NCORES = 8
BL = B // NCORES            # 512 rows per agent per core
R = NAG * BL                # 1536 rows per core
NB = BL                     # free-dim block = one agent block
P2 = 250                    # conv2 output positions
QT = 63                     # conv tiles of 4 positions (252 = 63*4)
NTOT = NAG * B

F32 = mybir.dt.float32
BF16 = mybir.dt.bfloat16
BF16NP = ml_dtypes.bfloat16
AX = mybir.AxisListType.X
AF = mybir.ActivationFunctionType
OP = mybir.AluOpType
LRELU = os.environ.get("V3_LRELU", "1") == "1"
ACCUM = os.environ.get("V3_ACCUM", "1") == "1"
SYNCDMA = os.environ.get("V3_SYNCDMA", "1") == "1"
TTR = os.environ.get("V3_TTR", "0") == "1"


def _sq_stats(nc, scr_ap, in_ap, accum_ap):
    """accum_ap = row-sums of in_ap**2 (scr_ap is scratch, same shape)."""
    if TTR:
        nc.vector.tensor_tensor_reduce(
            out=scr_ap, in0=in_ap, in1=in_ap,
            scale=1.0, scalar=0.0, op0=OP.mult, op1=OP.add,
            accum_out=accum_ap)
    else:
        nc.vector.tensor_mul(scr_ap, in_ap, in_ap)
        nc.vector.reduce_sum(accum_ap, scr_ap, axis=AX)


def _leaky_evict(nc, wpool, dst, src_ps, bias_ap):
    """dst = leaky_relu(src_ps + bias, 0.01), fused on ACT if available."""
    if LRELU:
        nc.scalar.activation(dst, src_ps, AF.Prelu, bias=bias_ap, alpha=0.01)
    else:
        tmp = wpool.tile([128, NB], F32, tag="lk")
        nc.scalar.activation(tmp[:], src_ps, AF.Identity, bias=bias_ap)
        nc.vector.scalar_tensor_tensor(
            out=dst, in0=tmp[:], scalar=0.01, in1=tmp[:],
            op0=OP.mult, op1=OP.max)


def _t1_parts(q):
    """conv1 tile q -> list of (t1 stack index, scan block index)."""
    if q <= 30:
        return [(q, 0)]
    if q == 31:
        return [(31, 0), (32, 1)]
    return [(q - 32, 1)]


def build_program():
    nc = bacc.Bacc(num_devices=NCORES)

    scan_t = nc.dram_tensor("scan_t", [SCAN, R], BF16, kind="ExternalInput")
    obs_t = nc.dram_tensor("obs_t", [STATE, R], BF16, kind="ExternalInput")
    acts_t = nc.dram_tensor("acts_t", [ACTD, R], BF16, kind="ExternalInput")
    t1_d = nc.dram_tensor("t1", [128, 33, 128], BF16, kind="ExternalInput")
    t2_d = nc.dram_tensor("t2", [128, 320], BF16, kind="ExternalInput")
    fc1w_d = nc.dram_tensor("fc1w", [128, QT, 256], BF16, kind="ExternalInput")
    fc2w_d = nc.dram_tensor("fc2w", [128, 2, 16], BF16, kind="ExternalInput")
    encw_d = nc.dram_tensor("encw", [128, 128], BF16, kind="ExternalInput")
    attw_d = nc.dram_tensor("attw", [128, 5, 128], BF16, kind="ExternalInput")
    hsum_d = nc.dram_tensor("hsum", [128, 8], BF16, kind="ExternalInput")
    hbc_d = nc.dram_tensor("hbc", [8, 128], BF16, kind="ExternalInput")
    mlpw_d = nc.dram_tensor("mlpw", [128, 2, 1152], BF16, kind="ExternalInput")
    bias_d = nc.dram_tensor("bias", [128, 20], F32, kind="ExternalInput")
    out_d = nc.dram_tensor("out", [2, R], F32, kind="ExternalOutput")

    with tile.TileContext(nc) as tc:
        with (
            tc.tile_pool(name="dram", bufs=1, space="DRAM") as dram,
            tc.tile_pool(name="cst", bufs=1) as cst,
            tc.tile_pool(name="ypool", bufs=7) as ypool,
            tc.tile_pool(name="opool", bufs=7) as opool,
            tc.tile_pool(name="xpool", bufs=4) as xpool,
            tc.tile_pool(name="qpool", bufs=3) as qpool,
            tc.tile_pool(name="othp", bufs=3) as othp,
            tc.tile_pool(name="attp", bufs=12) as attp,
            tc.tile_pool(name="lkp", bufs=3) as lkp,
            tc.tile_pool(name="mlph", bufs=26) as mlph,
        ):
            # ---- weight / input DMAs (program order ~ priority) ----
            # critical set first: bias, agent-2 scan, first conv weights
            biasb = cst.tile([128, 20], F32, tag="bias")
            nc.sync.dma_start(out=biasb, in_=bias_d[:])
            s0 = cst.tile([128, R], BF16, tag="s0")
            s1 = cst.tile([128, R], BF16, tag="s1")
            c2 = bass.ts(2, NB)
            for p in range(0, 128, 32):
                nc.sync.dma_start(out=s0[p:p + 32, c2],
                                  in_=scan_t[p:p + 32, c2])
            t1c = []
            for k in range(4):
                n = 9 if k < 3 else 6
                t = cst.tile([128, n, 128], BF16, tag=f"t1c{k}")
                t1c.append(t)
            for j in range(3):
                nc.sync.dma_start(out=t1c[0][:, j:j + 1, :],
                                  in_=t1_d[:, j:j + 1, :])
            t2sb = cst.tile([128, 320], BF16, tag="t2")
            nc.sync.dma_start(out=t2sb[:, 0:160], in_=t2_d[:, 0:160])
            nc.sync.dma_start(out=t2sb[:, 160:320], in_=t2_d[:, 160:320])
            fc1c = []
            for k in range(8):
                n = 8 if k < 7 else 7
                t = cst.tile([128, n, 256], BF16, tag=f"fc1c{k}")
                fc1c.append(t)
            for j in range(4):
                nc.sync.dma_start(out=fc1c[0][:, j:j + 1, :],
                                  in_=fc1w_d[:, j:j + 1, :])
            for j in range(3, 9):
                nc.sync.dma_start(out=t1c[0][:, j:j + 1, :],
                                  in_=t1_d[:, j:j + 1, :])
            for p in range(0, 128, 64):
                nc.sync.dma_start(out=s1[p:p + 64, c2],
                                  in_=scan_t[128 + p:128 + p + 64, c2])
            nc.sync.dma_start(out=fc1c[0][:, 4:8, :], in_=fc1w_d[:, 4:8, :])
            for k in range(1, 4):
                n = 9 if k < 3 else 6
                nc.sync.dma_start(out=t1c[k], in_=t1_d[:, 9 * k:9 * k + n, :])
            fc2w = cst.tile([128, 2, 16], BF16, tag="fc2w")
            nc.sync.dma_start(out=fc2w, in_=fc2w_d[:])
            nc.sync.dma_start(out=fc1c[1], in_=fc1w_d[:, 8:16, :])
            # agents 0/1 scan while agent-2 stream runs
            nc.sync.dma_start(out=s0[:, 0:NB], in_=scan_t[0:128, 0:NB])
            nc.sync.dma_start(out=s0[:, NB:2 * NB],
                              in_=scan_t[0:128, NB:2 * NB])
            nc.sync.dma_start(out=s1[:, 0:NB], in_=scan_t[128:256, 0:NB])
            nc.sync.dma_start(out=s1[:, NB:2 * NB],
                              in_=scan_t[128:256, NB:2 * NB])
            for k in range(2, 8):
                n = 8 if k < 7 else 7
                nc.sync.dma_start(out=fc1c[k], in_=fc1w_d[:, 8 * k:8 * k + n, :])
            # BN feature rows in 32-aligned groups: obs 0:48, feats 64:74,
            # acts 96:98; everything else memset to zero.
            inps = cst.tile([128, R], BF16, tag="inps")
            nc.vector.memset(inps[:], 0.0)
            nc.sync.dma_start(out=inps[0:STATE, :], in_=obs_t[:])
            nc.sync.dma_start(out=inps[96:96 + ACTD, :], in_=acts_t[:])
            encw = cst.tile([128, 128], BF16, tag="encw")
            nc.sync.dma_start(out=encw, in_=encw_d[:])
            attw = cst.tile([128, 5, 128], BF16, tag="attw")
            nc.sync.dma_start(out=attw, in_=attw_d[:])
            hsum = cst.tile([128, 8], BF16, tag="hsum")
            nc.sync.dma_start(out=hsum, in_=hsum_d[:])
            hbc = cst.tile([8, 128], BF16, tag="hbc")
            nc.sync.dma_start(out=hbc, in_=hbc_d[:])
            mlpw = cst.tile([128, 2, 1152], BF16, tag="mlpw")
            nc.sync.dma_start(out=mlpw, in_=mlpw_d[:])

            # ---- warm-up: one act-table load (sqrt set) + dummy CC ----
            scr = cst.tile([128, 2], F32, tag="scr")
            nc.vector.memset(scr[:], 0.0)
            scr2 = cst.tile([128, 1], F32, tag="scr2")
            nc.scalar.activation(scr2[:], scr[:, 0:1], AF.Sqrt)
            nc.scalar.activation(scr2[:], scr[:, 0:1], AF.Relu)
            ccd_in = dram.tile([1, 2], F32, tag="ccd_in")
            ccd_out = dram.tile([1, 2], F32, tag="ccd_out",
                                addr_space="Shared")
            nc.gpsimd.dma_start(out=ccd_in[:], in_=scr[0:1, :])
            nc.gpsimd.collective_compute(
                "AllReduce", OP.add,
                replica_groups=[list(range(NCORES))],
                ins=[ccd_in.opt()], outs=[ccd_out.opt()])

            saT = cst.tile([128, NAG, NB], BF16, tag="saT")
            keysT = cst.tile([128, NAG, NB], BF16, tag="keysT")
            valsT = cst.tile([128, NAG, NB], BF16, tag="valsT")
            outq1 = cst.tile([1, R], F32, tag="outq1")
            outq2 = cst.tile([1, R], F32, tag="outq2")
            # statsP/statsQ: cols 0-2 = per-agent early/full partials,
            # cols 3-4 = feats-row partials for agents 0/1 (rows 64:74)
            statsP = cst.tile([128, 5], F32, tag="statsP")
            statsQ = cst.tile([128, 5], F32, tag="statsQ")
            nc.vector.memset(statsP[:], 0.0)
            nc.vector.memset(statsQ[:], 0.0)
            sqscr = cst.tile([128, NB], F32, tag="sqscr")
            sqf = cst.tile([128, 2 * NB], F32, tag="sqf")
            stats2 = cst.tile([128, 2], F32, tag="stats2")

            # early obs/acts stats for agents 0,1 (feats rows still zero;
            # runs as soon as obs/acts DMAs land, hidden under the stream)
            for g in (0, 1):
                col = bass.ts(g, NB)
                nc.vector.reduce_sum(statsP[:, g:g + 1], inps[:, col],
                                     axis=AX)
                _sq_stats(nc, sqscr[:], inps[:, col], statsQ[:, g:g + 1])

            def evict_relu(dst, src_ps, bias_ap, use_act):
                if use_act:
                    nc.scalar.activation(dst, src_ps, AF.Relu, bias=bias_ap)
                else:
                    nc.vector.tensor_scalar(
                        out=dst, in0=src_ps, scalar1=bias_ap, scalar2=0.0,
                        op0=OP.add, op1=OP.max)

            # =========== conv stream (pre-BatchNorm) =========================
            with tc.tile_pool(name="ps_cv", bufs=4, space="PSUM") as ps_cv:
                def conv1(g, q, y_tiles, parity):
                    """conv1 for tile q of agent g -> y_tiles[q]."""
                    col = bass.ts(g, NB)
                    py = ps_cv.tile([128, NB], F32, tag="cnv", name=f"py{g}_{q}")
                    parts = _t1_parts(q)
                    for i, (idx, sb_) in enumerate(parts):
                        src = (s0 if sb_ == 0 else s1)[:, col]
                        nc.tensor.matmul(py, t1c[idx // 9][:, idx % 9, :],
                                         src, start=(i == 0),
                                         stop=(i == len(parts) - 1))
                    yq = ypool.tile([128, NB], BF16, tag="y")
                    evict_relu(yq[:], py[:], biasb[:, 0:1], parity)
                    y_tiles[q] = yq

                def conv1_pair(q, y0, y1, parity):
                    """conv1 tile q for agents 0 and 1, weight-major."""
                    py0 = ps_cv.tile([128, NB], F32, tag="cnv", name=f"pyA{q}")
                    py1 = ps_cv.tile([128, NB], F32, tag="cnv", name=f"pyB{q}")
                    parts = _t1_parts(q)
                    for i, (idx, sb_) in enumerate(parts):
                        src = (s0 if sb_ == 0 else s1)
                        lhs = t1c[idx // 9][:, idx % 9, :]
                        st, sp = (i == 0), (i == len(parts) - 1)
                        nc.tensor.matmul(py0, lhs, src[:, 0:NB], start=st,
                                         stop=sp, skip_group_check=True)
                        nc.tensor.matmul(py1, lhs, src[:, NB:2 * NB],
                                         start=st, stop=sp,
                                         skip_group_check=True)
                    ya = ypool.tile([128, NB], BF16, tag="y")
                    yb = ypool.tile([128, NB], BF16, tag="y")
                    evict_relu(ya[:], py0[:], biasb[:, 0:1], parity)
                    evict_relu(yb[:], py1[:], biasb[:, 0:1], not parity)
                    y0[q] = ya
                    y1[q] = yb

                def conv2(t, y_tiles_list, o2_list, parity):
                    """conv2 tile t for the given agents, weight-major."""
                    nag_ = len(y_tiles_list)
                    pos = [ps_cv.tile([128, NB], F32, tag="cnv",
                                      name=f"po{t}_{i}")
                           for i in range(nag_)]
                    if t < QT - 1:
                        rows, bcol = 128, 1
                        for i in range(nag_):
                            nc.tensor.matmul(pos[i], t2sb[:, 0:128],
                                             y_tiles_list[i][t][:],
                                             start=True, stop=False,
                                             skip_group_check=True)
                        for i in range(nag_):
                            nc.tensor.matmul(pos[i], t2sb[:, 128:256],
                                             y_tiles_list[i][t + 1][:],
                                             start=False, stop=True,
                                             skip_group_check=True)
                    else:
                        rows, bcol = 64, 17
                        for i in range(nag_):
                            nc.tensor.matmul(pos[i][0:64, :],
                                             t2sb[:, 256:320],
                                             y_tiles_list[i][t][:],
                                             start=True, stop=True)
                    for i in range(nag_):
                        o2 = opool.tile([128, NB], BF16, tag="o2")
                        evict_relu(o2[0:rows, :], pos[i][0:rows, :],
                                   biasb[0:rows, bcol:bcol + 1],
                                   (parity + i) % 2 == 0)
                        o2_list[i][t] = o2
                    for i in range(nag_):
                        if t > 0:
                            del y_tiles_list[i][t - 1]
                        if t == QT - 1:
                            del y_tiles_list[i][t]

                def fc1(t, o2_list, fc_ps_list):
                    """fc1 accumulation for tile t, weight-major."""
                    rows = 128 if t < QT - 1 else 64
                    for m in range(2):
                        lhs = fc1c[t // 8][0:rows, t % 8,
                                          128 * m:128 * m + 128]
                        for i in range(len(o2_list)):
                            nc.tensor.matmul(
                                fc_ps_list[i][m], lhs,
                                o2_list[i][t][0:rows, :],
                                start=(t == 0), stop=(t == QT - 1),
                                skip_group_check=True)
                    for i in range(len(o2_list)):
                        del o2_list[i][t]

                def fc1_finish(g, fc_ps, use_feats_accum):
                    """fc1 relu-evict, fc2, feats -> inps; stats partials."""
                    col = bass.ts(g, NB)
                    x3 = []
                    for m in range(2):
                        xm = xpool.tile([128, NB], BF16, tag="x3")
                        if m == 0:
                            nc.scalar.activation(xm[:], fc_ps[m][:], AF.Relu,
                                                 bias=biasb[:, 2:3])
                        else:
                            nc.vector.tensor_scalar(
                                out=xm[:], in0=fc_ps[m][:],
                                scalar1=biasb[:, 3:4], scalar2=0.0,
                                op0=OP.add, op1=OP.max)
                        x3.append(xm)
                    pf = ps_cv.tile([OUTF, NB], F32, tag="cnv",
                                    name=f"pf{g}")
                    for kb in range(2):
                        nc.tensor.matmul(pf, fc2w[:, kb, 0:OUTF], x3[kb][:],
                                         start=(kb == 0), stop=(kb == 1))
                    if use_feats_accum:
                        # agents 0/1 at stream end: feats-row stats on the
                        # least-loaded engine each (entry critical path)
                        if g == 0:
                            # ACT: evict g0; sums+sq via Square accum
                            nc.scalar.activation(inps[64:64 + OUTF, col],
                                                 pf[:], AF.Identity,
                                                 bias=biasb[0:OUTF, 4:5])
                            nc.scalar.activation(
                                sqf[64:64 + OUTF, 0:NB],
                                inps[64:64 + OUTF, col], AF.Square,
                                accum_out=statsQ[64:64 + OUTF, 3:4])
                            nc.vector.reduce_sum(
                                statsP[64:64 + OUTF, 3:4],
                                inps[64:64 + OUTF, col], axis=AX)
                        else:
                            # DVE: evict g1 + sums; gpsimd: sumsq
                            nc.vector.tensor_scalar(
                                out=inps[64:64 + OUTF, col], in0=pf[:],
                                scalar1=biasb[0:OUTF, 4:5], scalar2=None,
                                op0=OP.add)
                            nc.vector.reduce_sum(
                                statsP[64:64 + OUTF, 4:5],
                                inps[64:64 + OUTF, col], axis=AX)
                            nc.scalar.activation(
                                sqf[64:64 + OUTF, NB:2 * NB],
                                inps[64:64 + OUTF, col], AF.Square,
                                accum_out=statsQ[64:64 + OUTF, 4:5])
                    else:
                        # agent 2: full-column stats, hidden under 0/1 stream
                        nc.scalar.activation(inps[64:64 + OUTF, col], pf[:],
                                             AF.Identity,
                                             bias=biasb[0:OUTF, 4:5])
                        nc.vector.reduce_sum(statsP[:, g:g + 1],
                                             inps[:, col], axis=AX)
                        _sq_stats(nc, sqscr[:], inps[:, col],
                                  statsQ[:, g:g + 1])

                # ---------------- agent 2 solo ----------------
                with tc.tile_pool(name="ps_fA", bufs=2,
                                  space="PSUM") as ps_fA:
                    sc2 = nc.named_scope("conv_g2")
                    sc2.__enter__()
                    fcA = [ps_fA.tile([128, NB], F32, tag="pfc",
                                      name=f"pfcA{m}") for m in range(2)]
                    y2, oo2 = {}, {}
                    for q in range(QT):
                        conv1(2, q, y2, q % 2 == 0 and q >= 6)
                        if q >= 1:
                            conv2(q - 1, [y2], [oo2], q)
                        if q >= 2:
                            fc1(q - 2, [oo2], [fcA])
                    conv2(QT - 1, [y2], [oo2], 0)
                    fc1(QT - 2, [oo2], [fcA])
                    fc1(QT - 1, [oo2], [fcA])
                    fc1_finish(2, fcA, use_feats_accum=False)
                    sc2.__exit__(None, None, None)

                # ---------------- agents 0 + 1 interleaved ----------------
                with tc.tile_pool(name="ps_fB", bufs=4,
                                  space="PSUM") as ps_fB:
                    sc01 = nc.named_scope("conv_g01")
                    sc01.__enter__()
                    fcB = [[ps_fB.tile([128, NB], F32, tag="pfc",
                                       name=f"pfcB{g}{m}")
                            for m in range(2)] for g in range(2)]
                    y0, y1 = {}, {}
                    oA, oB = {}, {}
                    for q in range(QT):
                        conv1_pair(q, y0, y1, q % 2 == 0)
                        if q >= 1:
                            conv2(q - 1, [y0, y1], [oA, oB], q)
                        if q >= 2:
                            fc1(q - 2, [oA, oB], [fcB[0], fcB[1]])
                    conv2(QT - 1, [y0, y1], [oA, oB], 0)
                    fc1(QT - 2, [oA, oB], [fcB[0], fcB[1]])
                    fc1(QT - 1, [oA, oB], [fcB[0], fcB[1]])
                    fc1_finish(0, fcB[0], use_feats_accum=True)
                    fc1_finish(1, fcB[1], use_feats_accum=True)
                    sc01.__exit__(None, None, None)

                # ============ BatchNorm statistics + AllReduce ===========
                bn_scope = nc.named_scope("bn")
                bn_scope.__enter__()
                nc.vector.reduce_sum(stats2[:, 0:1], statsP[:], axis=AX)
                nc.vector.reduce_sum(stats2[:, 1:2], statsQ[:], axis=AX)
                cc_in = dram.tile([128, 2], F32, tag="cc_in")
                cc_out = dram.tile([128, 2], F32, tag="cc_out",
                                   addr_space="Shared")
                if SYNCDMA:
                    nc.sync.dma_start(out=cc_in[:], in_=stats2[:])
                else:
                    nc.gpsimd.dma_start(out=cc_in[:], in_=stats2[:])
                nc.gpsimd.collective_compute(
                    "AllReduce", OP.add,
                    replica_groups=[list(range(NCORES))],
                    ins=[cc_in.opt()], outs=[cc_out.opt()])
                gst = cst.tile([128, 2], F32, tag="gst")
                nc.sync.dma_start(out=gst[:], in_=cc_out[:])
                # PE filler: keep the HAM clock warm through the collective
                # (covers entry+algo latency every core pays; fillers have
                # no CC dependency so they run during the wait)
                nfill = int(os.environ.get("V3_FILL", "65"))
                if nfill:
                    fill_ps = [ps_cv.tile([128, NB], F32, tag="cnv",
                                          name=f"fill{i}") for i in range(2)]
                    for i in range(nfill):
                        nc.tensor.matmul(fill_ps[i % 2], t2sb[:, 0:128],
                                         s0[:, 0:NB], start=True, stop=True,
                                         skip_group_check=True)
                    nc.vector.tensor_scalar(out=scr[:, 0:2],
                                            in0=fill_ps[0][:, 0:2],
                                            scalar1=0.0, scalar2=None,
                                            op0=OP.mult)
                    nc.vector.tensor_scalar(out=scr[:, 0:2],
                                            in0=fill_ps[1][:, 0:2],
                                            scalar1=0.0, scalar2=None,
                                            op0=OP.mult)
            # var = gst1/N - (gst0/N)^2; sd = sqrt(var+eps)
            sqmu = cst.tile([128, 1], F32, tag="sqmu")
            nc.scalar.activation(sqmu[:], gst[:, 0:1], AF.Square,
                                 scale=1.0 / NTOT)
            var_ = cst.tile([128, 1], F32, tag="var_")
            nc.vector.scalar_tensor_tensor(
                out=var_[:], in0=gst[:, 1:2], scalar=1.0 / NTOT,
                in1=sqmu[:], op0=OP.mult, op1=OP.subtract)
            sd = cst.tile([128, 1], F32, tag="sd")
            nc.scalar.activation(sd[:], var_[:], AF.Sqrt,
                                 bias=biasb[:, 18:19])
            mu_bf = cst.tile([128, 1], BF16, tag="mu_bf")
            nc.scalar.activation(mu_bf[:], gst[:, 0:1], AF.Identity,
                                 scale=1.0 / NTOT)
            scr3 = cst.tile([1, 1], F32, tag="scr3")
            # fold BN into encoder: encw_s = encw / sd (per contraction
            # row); enc bias' = enc_b - encw_s.T @ mu
            rstd = cst.tile([128, 1], F32, tag="rstd")
            nc.vector.reciprocal(rstd[:], sd[:])
            encw_s = cst.tile([128, 128], BF16, tag="encw_s")
            nc.vector.tensor_scalar(out=encw_s[:], in0=encw[:],
                                    scalar1=rstd[:], scalar2=None,
                                    op0=OP.mult)
            encbias = cst.tile([128, 1], F32, tag="encbias")
            bn_scope.__exit__(None, None, None)

            # ======================= post-BN network =========================
            with (
                tc.tile_pool(name="ps_p", bufs=5, space="PSUM") as ps_p,
                tc.tile_pool(name="ps_l", bufs=1, space="PSUM") as ps_l,
                tc.tile_pool(name="ps_o", bufs=2, space="PSUM") as ps_o,
            ):
                post_scope = nc.named_scope("post")
                post_scope.__enter__()
                pmb = ps_l.tile([128, 1], F32, tag="pl", name="pmb")
                nc.tensor.matmul(pmb, encw_s[:], mu_bf[:],
                                 start=True, stop=True)
                nc.vector.tensor_tensor(out=encbias[:], in0=biasb[:, 5:6],
                                        in1=pmb[:], op=OP.subtract)

                # --- sa = leaky(encw_s @ inps + encbias), all agents ---
                pe_ = {}
                for g in (1, 2, 0):
                    p = ps_p.tile([128, NB], F32, tag="pp", name=f"pe{g}")
                    nc.tensor.matmul(p, encw_s[:], inps[:, bass.ts(g, NB)],
                                     start=True, stop=True)
                    pe_[g] = p
                for g in (1, 2, 0):
                    # fused bias + leaky relu (parametric_relu: every table)
                    _leaky_evict(nc, lkp, saT[:, g, :], pe_[g][:],
                                 encbias[:])


                # --- keys/vals/q; agent order 1,2,0 so that agent 0's
                # attention chain (which needs agents 1+2's keys/vals) can
                # start as early as possible ---
                ORD = (1, 2, 0)
                qT = {}
                pk_, pv_, pq_ = {}, {}, {}
                for g in ORD:
                    pk_[g] = ps_p.tile([128, NB], F32, tag="pp",
                                       name=f"pk{g}")
                    nc.tensor.matmul(pk_[g], attw[:, 0, :], saT[:, g, :],
                                     start=True, stop=True)
                for g in ORD:
                    nc.vector.tensor_scalar(out=keysT[:, g, :],
                                            in0=pk_[g][:], scalar1=0.0,
                                            scalar2=None, op0=OP.add)
                for g in ORD:
                    pv_[g] = ps_p.tile([128, NB], F32, tag="pp",
                                       name=f"pv{g}")
                    nc.tensor.matmul(pv_[g], attw[:, 1, :], saT[:, g, :],
                                     start=True, stop=True)
                for g in (0, 1, 2):
                    pq_[g] = ps_p.tile([128, NB], F32, tag="pp",
                                       name=f"pq{g}")
                    nc.tensor.matmul(pq_[g], attw[:, 2 + g, :], saT[:, g, :],
                                     start=True, stop=True)
                for g in ORD:
                    _leaky_evict(nc, lkp, valsT[:, g, :], pv_[g][:],
                                 biasb[:, 6:7])
                # sigmoid table load after the vals evicts, before the
                # first attention sigmoid needs it
                nc.scalar.activation(scr3[:], sd[0:1, 0:1], AF.Sigmoid)

                def q_evict(g):
                    qg = qpool.tile([128, NB], BF16, tag="qt", name=f"q{g}")
                    nc.vector.tensor_scalar(out=qg[:], in0=pq_[g][:],
                                            scalar1=0.0, scalar2=None,
                                            op0=OP.add)
                    qT[g] = qg

                # dk/dv/prod in dependency-first DVE order: agent 0's
                # chain only needs agents 1+2's keys/vals evictions
                dks, dvs, prods = {}, {}, {}

                def attn_dve(g):
                    oa, ob = [o for o in range(NAG) if o != g]
                    dk = attp.tile([128, NB], BF16, tag="dk", name=f"dk{g}")
                    nc.vector.tensor_sub(dk[:], keysT[:, oa, :],
                                         keysT[:, ob, :])
                    dv = attp.tile([128, NB], BF16, tag="dv", name=f"dv{g}")
                    nc.vector.tensor_sub(dv[:], valsT[:, oa, :],
                                         valsT[:, ob, :])
                    prod = attp.tile([128, NB], BF16, tag="prod",
                                     name=f"pr{g}")
                    nc.vector.tensor_mul(prod[:], qT[g][:], dk[:])
                    dks[g], dvs[g], prods[g] = dk, dv, prod

                q_evict(0)
                attn_dve(0)
                q_evict(1)
                attn_dve(1)
                q_evict(2)
                attn_dve(2)

                # --- attention per agent, MLP layer 1 interleaved ---
                h_cur = {}
                oth = {}

                def mlp_l1(g, net):
                    h_prev = (saT[:, g, :], oth[g][:])
                    h_new = []
                    for mb in range(2):
                        pm = ps_p.tile([128, NB], F32, tag="pp",
                                       name=f"pm{g}_{net}_0_{mb}")
                        for kb in range(2):
                            c0 = kb * 256 + mb * 128
                            nc.tensor.matmul(
                                pm, mlpw[:, net, c0:c0 + 128],
                                h_prev[kb], start=(kb == 0), stop=(kb == 1))
                        hm = mlph.tile([128, NB], BF16, tag="h")
                        bcol = (7 if net == 0 else 12) + mb
                        evict_relu(hm[:], pm[:], biasb[:, bcol:bcol + 1],
                                   (2 * g + net + mb) % 2 == 0)
                        h_new.append(hm)
                    h_cur[(g, net)] = tuple(h_new)

                for g in (0, 1, 2):
                    oa, ob = [o for o in range(NAG) if o != g]
                    dv = dvs[g]
                    pl = ps_l.tile([8, NB], F32, tag="pl")
                    nc.tensor.matmul(pl, hsum[:], prods[g][:],
                                     start=True, stop=True)
                    wa = attp.tile([8, NB], BF16, tag="wa")
                    nc.scalar.activation(wa[:], pl[:], AF.Sigmoid, scale=0.25)
                    pw = ps_p.tile([128, NB], F32, tag="pp", name=f"pw{g}")
                    nc.tensor.matmul(pw, hbc[:], wa[:], start=True, stop=True)
                    m1 = attp.tile([128, NB], F32, tag="m1")
                    nc.vector.tensor_mul(m1[:], pw[:], dv[:])
                    o = othp.tile([128, NB], BF16, tag="oth")
                    nc.vector.tensor_add(o[:], m1[:], valsT[:, ob, :])
                    oth[g] = o
                    mlp_l1(g, 0)
                    mlp_l1(g, 1)

                # --- MLP layer 2 weight-major (3-agent weight adjacency) ---
                h2 = {}
                for net in range(2):
                    pm2 = {}
                    for mb in range(2):
                        for g in range(NAG):
                            pm2[(g, mb)] = ps_p.tile(
                                [128, NB], F32, tag="pp",
                                name=f"pm2_{g}_{net}_{mb}")
                        for kb in range(2):
                            c0 = 512 + kb * 256 + mb * 128
                            lhs = mlpw[:, net, c0:c0 + 128]
                            for g in range(NAG):
                                nc.tensor.matmul(
                                    pm2[(g, mb)], lhs, h_cur[(g, net)][kb][:],
                                    start=(kb == 0), stop=(kb == 1),
                                    skip_group_check=True)
                        for g in range(NAG):
                            hm = mlph.tile([128, NB], BF16, tag="h")
                            bcol = (9 if net == 0 else 14) + mb
                            evict_relu(hm[:], pm2[(g, mb)][:],
                                       biasb[:, bcol:bcol + 1],
                                       (g + net + mb) % 2 == 0)
                            h2[(g, net, mb)] = hm

                # --- output layer, weight-major ---
                po_ = {}
                for net in range(2):
                    for kb in range(2):
                        lhs = mlpw[:, net, 1024 + 64 * kb:1025 + 64 * kb]
                        for g in range(NAG):
                            if kb == 0:
                                po_[(g, net)] = ps_o.tile(
                                    [1, NB], F32, tag="pout",
                                    name=f"po_{g}_{net}")
                            nc.tensor.matmul(
                                po_[(g, net)], lhs, h2[(g, net, kb)][:],
                                start=(kb == 0), stop=(kb == 1),
                                skip_group_check=True)
                for g in range(NAG):
                    col = bass.ts(g, NB)
                    for net in range(2):
                        bcol = 11 if net == 0 else 16
                        dst = (outq1 if net == 0 else outq2)[0:1, col]
                        nc.scalar.activation(dst, po_[(g, net)][:],
                                             AF.Identity,
                                             bias=biasb[0:1, bcol:bcol + 1])
                    nc.sync.dma_start(out=out_d[0:1, col],
                                      in_=outq1[0:1, col])
                    nc.sync.dma_start(out=out_d[1:2, col],
                                      in_=outq2[0:1, col])
                post_scope.__exit__(None, None, None)
    return nc


# ======================= host-side preparation ===========================

def _prep_shared(i):
    f32 = np.float32
    w1 = np.asarray(i["conv_w1"], f32)[:, 0, :]           # [32, 5]
    w2 = np.asarray(i["conv_w2"], f32)                    # [32, 32, 3]
    fw1 = np.asarray(i["fc_w1"], f32)                     # [256, 8000]
    fw2 = np.asarray(i["fc_w2"], f32)                     # [10, 256]
    encw_ = np.asarray(i["enc_w"], f32)                   # [128, 60]
    Wk = np.asarray(i["Wk"], f32)
    Wv = np.asarray(i["Wv"], f32)
    Wq = np.asarray(i["Wq"], f32)

    t1 = np.zeros((128, 33, 128), f32)
    for idx in range(32):
        r0 = 4 * idx if idx < 31 else 124
        for dp in range(4):
            for j in range(5):
                r = r0 + dp + j
                if r < 128:
                    t1[r, idx, dp::4] = w1[:, j]
    for dp in range(4):
        for r in range(4):
            j = r + 4 - dp
            if 0 <= j < 5:
                t1[r, 32, dp::4] = w1[:, j]

    t2 = np.zeros((128, 320), f32)
    for dp in range(4):
        for j in range(3):
            e = dp + j
            for cp in range(32):
                if e < 4:
                    t2[4 * cp + e, dp:128:4] = w2[:, cp, j]
                else:
                    t2[4 * cp + (e - 4), 128 + dp:256:4] = w2[:, cp, j]
    for dp in range(2):
        for j in range(3):
            e = dp + j
            for cp in range(32):
                t2[4 * cp + e, 256 + dp:320:2] = w2[:, cp, j]

    fc1w = np.zeros((128, QT, 256), f32)
    for q in range(QT - 1):
        for dp in range(4):
            p = 4 * q + dp
            fc1w[dp::4, q, :] = fw1[:, p::P2].T
    for dp in range(2):
        fc1w[dp:64:2, QT - 1, :] = fw1[:, (248 + dp)::P2].T

    fc2w = np.zeros((128, 2, 16), f32)
    for kb in range(2):
        fc2w[:, kb, 0:OUTF] = fw2[:, 128 * kb:128 * kb + 128].T

    encw_full = np.zeros((128, 128), f32)
    encw_full[0:STATE, :] = encw_.T[0:STATE, :]            # obs rows
    encw_full[64:64 + OUTF, :] = encw_.T[50:60, :]         # feats rows
    encw_full[96:96 + ACTD, :] = encw_.T[48:50, :]         # acts rows

    attw = np.zeros((128, 5, 128), f32)
    attw[:, 0, :] = Wk.reshape(128, H).T
    attw[:, 1, :] = Wv.reshape(128, H).T
    for g in range(NAG):
        attw[:, 2 + g, :] = Wq[g].reshape(128, H).T

    hsum = np.kron(np.eye(8, dtype=f32), np.ones((16, 1), f32))  # [128, 8]
    hbc = np.ascontiguousarray(hsum.T)                           # [8, 128]

    mlpw = np.zeros((128, 2, 1152), f32)
    for net, pre in enumerate(("q1", "q2")):
        mw1 = np.asarray(i[pre + "_w1"], f32)
        mw2 = np.asarray(i[pre + "_w2"], f32)
        mw3 = np.asarray(i[pre + "_w3"], f32)
        for kb in range(2):
            mlpw[:, net, kb * 256:(kb + 1) * 256] = \
                mw1[:, 128 * kb:128 * kb + 128].T
            mlpw[:, net, 512 + kb * 256:512 + (kb + 1) * 256] = \
                mw2[:, 128 * kb:128 * kb + 128].T
            mlpw[:, net, 1024 + 64 * kb] = mw3[0, 128 * kb:128 * kb + 128]

    bias = np.zeros((128, 20), f32)
    bias[:, 0] = np.repeat(np.asarray(i["conv_b1"], f32), 4)
    bias[:, 1] = np.repeat(np.asarray(i["conv_b2"], f32), 4)
    bias[:, 2] = np.asarray(i["fc_b1"], f32)[0:128]
    bias[:, 3] = np.asarray(i["fc_b1"], f32)[128:256]
    bias[0:OUTF, 4] = np.asarray(i["fc_b2"], f32)
    bias[:, 5] = np.asarray(i["enc_b"], f32)
    bias[:, 6] = np.asarray(i["bv"], f32).reshape(128)
    for net, pre in enumerate(("q1", "q2")):
        b1 = np.asarray(i[pre + "_b1"], f32)
        b2 = np.asarray(i[pre + "_b2"], f32)
        b3 = np.asarray(i[pre + "_b3"], f32)
        c0 = 7 if net == 0 else 12
        bias[:, c0] = b1[0:128]
        bias[:, c0 + 1] = b1[128:256]
        bias[:, c0 + 2] = b2[0:128]
        bias[:, c0 + 3] = b2[128:256]
        bias[0, 11 if net == 0 else 16] = b3[0]
    bias[0:64, 17] = np.repeat(np.asarray(i["conv_b2"], f32), 2)
    bias[:, 18] = EPS

    bf = BF16NP
    return {
        "t1": t1.astype(bf), "t2": t2.astype(bf),
        "fc1w": fc1w.astype(bf), "fc2w": fc2w.astype(bf),
        "encw": encw_full.astype(bf), "attw": attw.astype(bf),
        "hsum": hsum.astype(bf), "hbc": hbc.astype(bf),
        "mlpw": mlpw.astype(bf), "bias": bias,
    }


def _shard(arr, c):
    out = np.empty((R, arr.shape[1]), np.float32)
    for g in range(NAG):
        out[g * BL:(g + 1) * BL] = arr[g * B + c * BL: g * B + (c + 1) * BL]
    return np.ascontiguousarray(out.T).astype(BF16NP)


_CACHE = {}


def _strip_redundant_ldweights(nc):
    """Remove back-to-back InstLdweights with identical weight APs.

    The PE keeps its stationary operand between matmuls; a reload of the
    same weights forces the array to drain first (~50ns/matmul measured).
    Runs pre-finalize (waits are still on the matmuls at this point);
    references to a removed load are remapped to the kept one.
    """
    removed = 0
    mapping = {}
    for f in nc.m.functions:
        for b in f.blocks:
            insts = list(b.instructions)
            out, last_sig, kept_name = [], None, None
            for inst in insts:
                if type(inst).__name__ == 'InstLdweights':
                    sig = (str(inst.ins[0]) + '|' + str(inst.perf_mode) +
                           '|' + str(inst.is_transpose) + '|' +
                           str(inst.tile_position))
                    if sig == last_sig and kept_name is not None:
                        mapping[inst.name] = kept_name
                        removed += 1
                        continue
                    last_sig, kept_name = sig, inst.name
                out.append(inst)
            if len(out) != len(insts):
                b.instructions = out
    if mapping:
        for f in nc.m.functions:
            for b in f.blocks:
                for inst in b.instructions:
                    inst.remap_dependency_names(mapping)
    return removed


def _get_prog():
    if "nc" not in _CACHE:
        nc = build_program()
        if os.environ.get("V3_STRIP", "1") == "1":
            _strip_redundant_ldweights(nc)
        nc.finalize()
        _CACHE["nc"] = nc
    return _CACHE["nc"]


def _make_in_maps(inputs):
    shared = _prep_shared(inputs)
    obs = np.asarray(inputs["obs"], np.float32)
    acts = np.asarray(inputs["acts"], np.float32)
    scan = np.asarray(inputs["scan"], np.float32)
    in_maps = []
    for c in range(NCORES):
        m = dict(shared)
        m["scan_t"] = _shard(scan, c)
        m["obs_t"] = _shard(obs, c)
        m["acts_t"] = _shard(acts, c)
        in_maps.append(m)
    return in_maps


def _gather(results):
    q = np.empty((2, NAG, B), np.float32)
    for c, r in enumerate(results):
        o = np.asarray(r["out"]).reshape(2, NAG, BL)
        q[:, :, c * BL:(c + 1) * BL] = o
    q1 = np.ascontiguousarray(q[0].reshape(NTOT, 1))
    q2 = np.ascontiguousarray(q[1].reshape(NTOT, 1))
    return q1, q2


def kernel(**inputs):
    nc = _get_prog()
    in_maps = _make_in_maps(inputs)
    if os.environ.get("KERNEL_BACKEND") == "sim":
        from concourse import bass_interp
        sim = bass_interp.MultiCoreSim(nc, NCORES)
        for c in range(NCORES):
            for k, v in in_maps[c].items():
                sim.cores[c].tensor(k)[:] = v
        sim.simulate()
        results = [{"out": np.array(sim.cores[c].tensor("out"))}
                   for c in range(NCORES)]
        return _gather(results)
    res = run_bass_kernel_spmd(nc, in_maps, list(range(NCORES)))
    return _gather(res.results)


def kernel_profiled(**inputs):
    """Like kernel() but NTFF-traced; returns ((q1, q2), exec_time_ns)."""
    _install_ntff_hook()
    nc = _get_prog()
    in_maps = _make_in_maps(inputs)
    res = run_bass_kernel_spmd(nc, in_maps, list(range(NCORES)), trace=True)
    return _gather(res.results), res.exec_time_ns


def _install_ntff_hook():
    """Provide antenv.axon_hooks (absent in this image) and register the
    ctypes NTFF profile hook against libaxon_pjrt.so."""
    import sys
    import types
    import ctypes
    import contextlib

    if "antenv.axon_hooks" not in sys.modules:
        mod = types.ModuleType("antenv.axon_hooks")
        mod._hook = None
        mod.set_axon_ntff_profile_hook = lambda h: setattr(mod, "_hook", h)
        mod.get_axon_ntff_profile_hook = lambda: mod._hook
        sys.modules["antenv.axon_hooks"] = mod
        import antenv
        antenv.axon_hooks = mod
    mod = sys.modules["antenv.axon_hooks"]
    if mod.get_axon_ntff_profile_hook() is not None:
        return
    so_path = "/opt/axon/libaxon_pjrt.so"
    lib = ctypes.CDLL(so_path)
    if not hasattr(lib, "axon_start_nrt_profile"):
        return
    lib.axon_start_nrt_profile.argtypes = [
        ctypes.POINTER(ctypes.c_int64), ctypes.c_size_t]
    lib.axon_start_nrt_profile.restype = ctypes.c_int64
    lib.axon_stop_nrt_profile.argtypes = [ctypes.c_char_p]
    lib.axon_stop_nrt_profile.restype = ctypes.c_int64

    @contextlib.contextmanager
    def _hook(output_dir, device_ids):
        import jax
        jax.devices()
        if device_ids:
            ids = (ctypes.c_int64 * len(device_ids))(*device_ids)
            rc = lib.axon_start_nrt_profile(ids, len(device_ids))
        else:
            rc = lib.axon_start_nrt_profile(None, 0)
        if rc != 0:
            raise RuntimeError(f"axon_start_nrt_profile rc={rc}")
        try:
            yield
        finally:
            n = lib.axon_stop_nrt_profile(str(output_dir).encode())
            if n < 0:
                raise RuntimeError(f"axon_stop_nrt_profile rc={n}")

    mod.set_axon_ntff_profile_hook(_hook)
